# revision 9
# baseline (speedup 1.0000x reference)
"""Trainium2 Bass kernel for nn_BiAttention.

Data-parallel over batch across 8 NeuronCores (2 batches per core).

Per batch (QL=512, CL=2048, D=2048), with S[i,j] = sc[i] + sq[j] + G[i,j],
G = (c*wm) @ q^T:
  - alpha = softmax_j(S): sc[i] cancels in the row softmax, so we
    exponentiate E[j,i] = exp(G^T[j,i] + sq[j]) (values O(1)) and
    normalize lazily via r[i] = 1/sum_j E.
  - context2question = (E^T @ q) * r; the r multiply rides the PSUM
    evacuation (ACT copy with per-partition scale), the c multiply rides
    the following DVE op.
  - beta = softmax_i(max_j S): b[i] = (max_j E[j,i]) * exp(sc[i]),
    q2c = (b @ c) / sum(b); host finishes q2c/zb and the last third.

v4 vs v3:
  - M1 (G = qm @ c^T) and M2 (c2q = alpha^T @ q) run in fp8e4m3 with
    perf_mode=DoubleRow: operand pair-tiles [P, 2, N] contract 256 rows
    per matmul, halving both the matmul count and the streamed cycles.
    qm is prescaled by 64 so wm*q values sit in fp8's normal range; the
    exp undoes it via scale=1/64.  E stays UNNORMALIZED in fp8 (alpha
    values ~2e-3 would be fp8-subnormal); r is applied at O2 evac time
    as a per-partition ACT scale (r transposed to columns by K=1
    matmuls against a [1,1] ones vector).
  - z and sc contractions also use DoubleRow (fp8 pair weights padded to
    16-byte strides).
  - Output stores moved to the SP DGE queue (ACT now does exp + evac).
"""

from contextlib import ExitStack

import numpy as np

import concourse.bass as bass
import concourse.mybir as mybir
import concourse.tile as tile
from concourse import bacc
from concourse.bass import ts
from concourse.bass_utils import run_bass_kernel_spmd
from concourse.masks import make_identity

P = 128
B, QL, CL, D = 16, 512, 2048, 2048
N_CORES = 8
B_LOC = B // N_CORES

F32 = mybir.dt.float32
BF16 = mybir.dt.bfloat16
FP8 = mybir.dt.float8e4
AX = mybir.AxisListType
ALU = mybir.AluOpType
ACTF = mybir.ActivationFunctionType
DR = mybir.MatmulPerfMode.DoubleRow

QSCALE = 64.0


CFG = {
    "outp_bufs": 2,
    "e_bufs": 2,
    "mini_bufs": 2,
    "work_bufs": 2,
    "mm_bufs": 4,
    "tp_bufs": 2,
    "ct_act_mod": 2,        # d % mod != 0 -> ACT, else DVE for cT evac
    "st_eng": "sp",         # output-store DGE queue
    "cast_dma": True,       # SWDGE cast loads
}


def pair2(t, width):
    """View a [P, 2*width] tile as [P, 2, width]."""
    return t.rearrange("p (o w) -> p o w", o=2)


def emit(ctx, nc, tc, q_in, c_in, w_in, out, q2c_out, zb_out,
         B_loc, QLd, CLd, Dd):
    """Emit the Tile program. Dimensions parameterized for small-shape sim."""
    JC = QLd // P          # j-chunks (4)
    KJ = JC // 2           # j-chunk pairs (2)
    DC = Dd // P           # d-chunks (16)
    KD = DC // 2           # d-chunk pairs (8)
    IC = CLd // P          # i-chunks (16)
    IBW = min(512, CLd)    # i-block width
    IB = IBW // P          # i-chunks per block (4)
    NBLK = IC // IB        # i-blocks per batch (4)
    DBW = min(512, Dd)     # d-block width for M2
    DB = Dd // DBW         # d-blocks (4)

    const = ctx.enter_context(tc.tile_pool(name="const", bufs=1))
    io = ctx.enter_context(tc.tile_pool(name="io", bufs=3))
    res = ctx.enter_context(tc.tile_pool(name="res", bufs=1))
    ctp = ctx.enter_context(tc.tile_pool(name="ctp", bufs=1))
    work = ctx.enter_context(tc.tile_pool(name="work", bufs=CFG["work_bufs"]))
    work1 = ctx.enter_context(tc.tile_pool(name="work1", bufs=1))
    outp = ctx.enter_context(tc.tile_pool(name="outp", bufs=CFG["outp_bufs"]))
    small = ctx.enter_context(tc.tile_pool(name="small", bufs=2))
    rows = ctx.enter_context(tc.tile_pool(name="rows", bufs=1))
    ps_big = ctx.enter_context(tc.tile_pool(name="psb", bufs=2, space="PSUM"))
    ps_min = ctx.enter_context(tc.tile_pool(name="psm", bufs=1, space="PSUM"))

    # ---- constants ----
    wcol = const.tile([P, 3 * DC], F32)      # (p, col) = wsim[col*128 + p]
    nc.sync.dma_start(wcol, w_in.rearrange("(o p) -> p o", p=P))
    wm64 = const.tile([P, DC], F32)          # 64 * wm, per-partition scales
    nc.vector.tensor_scalar(wm64, wcol[:, 2 * DC:3 * DC], QSCALE, None,
                            ALU.mult)
    wqf = io.tile([P, Dd], F32, tag="fio", name="wqf")
    nc.sync.dma_start(wqf, w_in[None, 0:Dd].to_broadcast([P, Dd]))
    wq_bc = const.tile([P, Dd], BF16)        # wq broadcast to all partitions
    nc.vector.tensor_copy(wq_bc, wqf)
    ident = const.tile([P, P], BF16)
    make_identity(nc, ident)
    ident8 = const.tile([P, P], FP8)
    nc.vector.tensor_copy(ident8, ident)
    ones_col = const.tile([P, 1], BF16)
    nc.vector.memset(ones_col, 1.0)
    ones_col_f = const.tile([P, 1], F32)
    nc.vector.memset(ones_col_f, 1.0)
    one_f = const.tile([1, 1], F32)
    nc.vector.memset(one_f, 1.0)
    ones_col8 = const.tile([P, 1], FP8)
    nc.vector.memset(ones_col8, 1.0)
    wc64_f8 = const.tile([P, DC], FP8)       # 64 * wc columns
    nc.vector.tensor_scalar(wc64_f8, wcol[:, DC:2 * DC], QSCALE, None,
                            ALU.mult)

    def load_cast(t, src_ap, dtype):
        if CFG["cast_dma"]:
            nc.gpsimd.dma_start(t, src_ap)
        else:
            f = io.tile([P, src_ap.shape[-1]], F32, tag="fio", name="fio")
            nc.sync.dma_start(f, src_ap)
            nc.scalar.copy(t, f)

    def q_phase(b):
        # Q phase: load q as fp8 pair-tiles, sq, transposed 64*wm-scaled qmT
        pe = b % 2
        qb2 = []
        for kj in range(KJ):
            t = res.tile([P, 2 * Dd], FP8, tag=f"qb{pe}_{kj}",
                         name=f"qb{pe}_{kj}")
            for o in range(2):
                load_cast(t[:, o * Dd:(o + 1) * Dd],
                          q_in[b, ts(2 * kj + o, P), :], FP8)
            qb2.append(t)
        sq = []
        for j in range(JC):
            scr = work1.tile([P, Dd], BF16, tag="ttr_scr", name="ttr_scr")
            s = small.tile([P, 1], F32, tag=f"sq{pe}_{j}", name=f"sq{pe}_{j}")
            nc.vector.tensor_mul(
                scr, qb2[j // 2][:, (j % 2) * Dd:(j % 2 + 1) * Dd], wq_bc)
            nc.vector.reduce_sum(s, scr, axis=AX.X)
            sq.append(s)
        qmT2 = []
        for k in range(KD):
            qmT2.append(res.tile([P, 2 * QLd], FP8, tag=f"qmT{pe}_{k}",
                                 name=f"qmT{pe}_{k}"))
        for d in range(DC):
            # fp8 PE transpose writes 16-bit granules: output element step
            # must be 2, so transpose into a stride-2 view and read it back
            # strided during the scale-evacuation.
            tp = ps_big.tile([P, 2 * QLd], FP8, tag="tp", name="tp",
                             bufs=CFG["tp_bufs"])
            tpv = tp.rearrange("p (q two) -> p q two", two=2)
            for j in range(JC):
                nc.tensor.transpose(
                    tpv[:, ts(j, P), 0:1],
                    qb2[j // 2][:, (j % 2) * Dd + d * P:
                                (j % 2) * Dd + (d + 1) * P],
                    ident8)
            dst = qmT2[d // 2][:, (d % 2) * QLd:(d % 2 + 1) * QLd]
            nc.vector.tensor_scalar(dst, tpv[:, :, 0:1], wm64[:, d:d + 1],
                                    None, ALU.mult)
        return qb2, sq, qmT2

    for b in range(B_loc):
        qb2, sq, qmT2 = q_phase(b)

        b_all = small.tile([P, IC], BF16, tag=f"ball{b % 2}", name="ball")
        q2c_acc = rows.tile([1, Dd], F32, tag="q2ca", name="q2ca")
        cb = [None] * IC

        for blk in range(NBLK):
            # ---- load c rows (bf16 cast in DMA) ----
            for ii in range(IB):
                ig = blk * IB + ii
                t = res.tile([P, Dd], BF16, tag=f"cb{ig}", name=f"cb{ig}")
                load_cast(t, c_in[b, ts(ig, P), :], BF16)
                cb[ig] = t
            # ---- transpose block of c -> fp8 pair tiles ----
            cT2 = []
            for k in range(KD):
                cT2.append(ctp.tile([P, 2 * IBW], FP8, tag=f"ct{k}",
                                    name=f"ct{k}"))
            for d in range(DC):
                tp = ps_big.tile([P, IBW], BF16, tag="tp", name="tp",
                                 bufs=CFG["tp_bufs"])
                for ii in range(IB):
                    nc.tensor.transpose(tp[:, ts(ii, P)],
                                        cb[blk * IB + ii][:, ts(d, P)], ident)
                dst = cT2[d // 2][:, (d % 2) * IBW:(d % 2 + 1) * IBW]
                if d % CFG["ct_act_mod"] == 0:
                    nc.vector.tensor_copy(dst, tp)
                else:
                    nc.scalar.copy(dst, tp)
            # ---- M1 (DoubleRow fp8): 64*(G^T + .) then exp(scale=1/64) ----
            E2 = []
            for kj in range(KJ):
                E2.append(work.tile([P, 2 * IBW], FP8, tag=f"e{kj}",
                                    name=f"e{kj}", bufs=CFG["e_bufs"]))
            for j in range(JC):
                m1 = ps_big.tile([P, IBW], F32, tag="mm", name="m1",
                                 bufs=CFG["mm_bufs"])
                for k in range(KD):
                    nc.tensor.matmul(
                        m1,
                        lhsT=pair2(qmT2[k], QLd)[:, :, ts(j, P)],
                        rhs=pair2(cT2[k], IBW),
                        start=(k == 0), stop=(k == KD - 1), perf_mode=DR)
                nc.scalar.activation(
                    E2[j // 2][:, (j % 2) * IBW:(j % 2 + 1) * IBW], m1,
                    ACTF.Exp, bias=sq[j], scale=1.0 / QSCALE)
            # ---- Z row; r = 1/Z; transpose r to columns via K=1 matmuls ----
            z = ps_min.tile([1, IBW], F32, tag="mini", name="mini",
                            bufs=CFG["mini_bufs"])
            for j in range(JC):
                nc.tensor.matmul(
                    z, lhsT=ones_col8,
                    rhs=E2[j // 2][:, (j % 2) * IBW:(j % 2 + 1) * IBW],
                    start=(j == 0), stop=(j == JC - 1))
            r_row = small.tile([1, IBW], F32, tag="rrow", name="rrow", bufs=1)
            nc.vector.reciprocal(r_row, z)
            rcp = ps_min.tile([P, IB], F32, tag="mini", name="rcp",
                              bufs=CFG["mini_bufs"])
            for ii in range(IB):
                nc.tensor.matmul(rcp[:, ii:ii + 1],
                                 lhsT=r_row[0:1, ts(ii, P)], rhs=one_f,
                                 start=True, stop=True)
            r_cols = small.tile([P, IB], F32, tag="rcols", name="rcols")
            nc.vector.tensor_copy(r_cols, rcp)
            # ---- max over j (partition dim, unnormalized E) ----
            mx = work.tile([P, IBW], FP8, tag="mx", name="mx")
            nc.vector.tensor_copy(mx, E2[0][:, 0:IBW])
            for kj in range(KJ):
                for o in range(2):
                    if kj == 0 and o == 0:
                        continue
                    nc.vector.tensor_max(
                        mx, mx, E2[kj][:, o * IBW:(o + 1) * IBW])
            # ---- sc row (DoubleRow vs cT2), b = exp(sc) * max_j E ----
            scp = ps_min.tile([1, IBW], F32, tag="mini", name="mini",
                              bufs=CFG["mini_bufs"])
            for d in range(DC):
                nc.tensor.matmul(
                    scp, lhsT=wc64_f8[:, d:d + 1],
                    rhs=cT2[d // 2][:, (d % 2) * IBW:(d % 2 + 1) * IBW],
                    start=(d == 0), stop=(d == DC - 1))
            escrow = small.tile([1, IBW], FP8, tag="escrow", name="escrow",
                                bufs=1)
            nc.scalar.activation(escrow, scp, ACTF.Exp, scale=1.0 / QSCALE)
            esc_bc = work.tile([P, IBW], FP8, tag="escbc", name="escbc")
            nc.gpsimd.partition_broadcast(esc_bc, escrow)
            nc.vector.tensor_mul(mx, mx, esc_bc)
            mx_b = work.tile([P, IBW], BF16, tag="mxb", name="mxb")
            nc.vector.tensor_copy(mx_b, mx)
            mtp = ps_big.tile([P, IBW], BF16, tag="tp", name="mtp",
                              bufs=CFG["tp_bufs"])
            for ii in range(IB):
                nc.tensor.transpose(mtp[:, ts(ii, P)], mx_b[:, ts(ii, P)],
                                    ident)
            for ii in range(IB):
                nc.vector.reduce_max(b_all[:, blk * IB + ii:blk * IB + ii + 1],
                                     mtp[:, ts(ii, P)], axis=AX.X)
            # ---- q2c partial sums (bf16, contract i within block) ----
            for db in range(DB):
                qp = ps_min.tile([1, DBW], F32, tag="mini", name="mini",
                                 bufs=CFG["mini_bufs"])
                for ii in range(IB):
                    ig = blk * IB + ii
                    nc.tensor.matmul(qp, lhsT=b_all[:, ig:ig + 1],
                                     rhs=cb[ig][:, ts(db, DBW)],
                                     start=(ii == 0), stop=(ii == IB - 1))
                a_sl = q2c_acc[0:1, ts(db, DBW)]
                if blk == 0:
                    nc.vector.tensor_copy(a_sl, qp)
                else:
                    nc.vector.tensor_add(a_sl, qp, a_sl)

            # ---- M2 (DoubleRow fp8) + O2 = (u * r) * c via ACT+DVE ----
            for ii in range(IB):
                ig = blk * IB + ii
                o2 = outp.tile([P, Dd], BF16, tag="ob2", name="o2")
                us = [ps_big.tile([P, DBW], F32, tag="mm", name=f"u{db}",
                                  bufs=CFG["mm_bufs"]) for db in range(DB)]
                for kj in range(KJ):
                    for db in range(DB):
                        nc.tensor.matmul(
                            us[db],
                            lhsT=pair2(E2[kj], IBW)[:, :, ts(ii, P)],
                            rhs=pair2(qb2[kj], Dd)[:, :, ts(db, DBW)],
                            start=(kj == 0), stop=(kj == KJ - 1), perf_mode=DR)
                for db in range(DB):
                    tmp = work.tile([P, DBW], BF16, tag="o2t", name="o2t",
                                    bufs=CFG.get("o2t_bufs", 3))
                    nc.scalar.activation(tmp, us[db], ACTF.Copy, bias=0.0,
                                         scale=r_cols[:, ii:ii + 1])
                    nc.vector.tensor_mul(o2[:, ts(db, DBW)], tmp,
                                         cb[ig][:, ts(db, DBW)])
                st = nc.scalar if CFG["st_eng"] == "act" else nc.sync
                st.dma_start(out[b, ts(ig, P), 0:Dd], o2)

        # ---- finalize beta-sum + q2c row; host does division and O3 ----
        bs = small.tile([P, 1], F32, tag="bsum", name="bsum")
        nc.vector.reduce_sum(bs, b_all[:, 0:IC], axis=AX.X)
        zb = ps_min.tile([1, 1], F32, tag="mini", name="mini",
                         bufs=CFG["mini_bufs"])
        nc.tensor.matmul(zb, lhsT=ones_col_f, rhs=bs)
        zb_sb = small.tile([1, 1], F32, tag="zbsb", name="zbsb")
        nc.vector.tensor_copy(zb_sb, zb)
        st = nc.scalar if CFG["st_eng"] == "act" else nc.sync
        st.dma_start(zb_out[b, None, 0:1], zb_sb[0:1, 0:1])
        st.dma_start(q2c_out[b, None, :], q2c_acc)


def build(B_loc=B_LOC, QLd=QL, CLd=CL, Dd=D):
    nc = bacc.Bacc("TRN2", target_bir_lowering=False, debug=False,
                   enable_asserts=False, num_devices=1)
    q_in = nc.dram_tensor("question", [B_loc, QLd, Dd], F32,
                          kind="ExternalInput").ap()
    c_in = nc.dram_tensor("context", [B_loc, CLd, Dd], F32,
                          kind="ExternalInput").ap()
    w_in = nc.dram_tensor("wsim", [3 * Dd], F32, kind="ExternalInput").ap()
    out = nc.dram_tensor("out", [B_loc, CLd, Dd], BF16,
                         kind="ExternalOutput").ap()
    q2c_out = nc.dram_tensor("q2c", [B_loc, Dd], F32,
                             kind="ExternalOutput").ap()
    zb_out = nc.dram_tensor("zb", [B_loc, 1], F32,
                            kind="ExternalOutput").ap()
    with tile.TileContext(nc) as tc, ExitStack() as ctx:
        emit(ctx, nc, tc, q_in, c_in, w_in, out, q2c_out, zb_out,
             B_loc, QLd, CLd, Dd)
    nc.compile()
    return nc


_CACHED_NC = None


def _get_nc():
    global _CACHED_NC
    if _CACHED_NC is None:
        _CACHED_NC = build()
    return _CACHED_NC


def _shard(question, context, wsim):
    in_maps = []
    for i in range(N_CORES):
        in_maps.append({
            "question": np.ascontiguousarray(question[i * B_LOC:(i + 1) * B_LOC],
                                             dtype=np.float32),
            "context": np.ascontiguousarray(context[i * B_LOC:(i + 1) * B_LOC],
                                            dtype=np.float32),
            "wsim": np.ascontiguousarray(wsim, dtype=np.float32),
        })
    return in_maps


def kernel_raw(question, context, wsim, **run_kwargs):
    """Run and return the full BassKernelResults (for profiling)."""
    nc = _get_nc()
    in_maps = _shard(np.asarray(question), np.asarray(context),
                     np.asarray(wsim))
    res = run_bass_kernel_spmd(nc, in_maps, core_ids=list(range(N_CORES)),
                               **run_kwargs)
    return res


def kernel(question, context, wsim):
    res = kernel_raw(question, context, wsim)
    context = np.asarray(context, np.float32)
    out2 = np.concatenate([np.asarray(res.results[i]["out"])
                           for i in range(N_CORES)], axis=0)  # (B, CL, D) bf16
    q2c = np.concatenate([np.asarray(res.results[i]["q2c"])
                          for i in range(N_CORES)], axis=0)   # (B, D) f32
    zb = np.concatenate([np.asarray(res.results[i]["zb"])
                         for i in range(N_CORES)], axis=0)    # (B, 1) f32
    q2c_row = (q2c / zb).astype(np.float32)                   # (B, D)
    full = np.empty((B, CL, 3 * D), np.float32)
    full[:, :, :D] = context                                  # exact
    full[:, :, D:2 * D] = out2.astype(np.float32)
    full[:, :, 2 * D:] = context * q2c_row[:, None, :]
    return full
